# revision 8
# baseline (speedup 1.0000x reference)
"""Trainium2 Bass kernel for nn_Net_39230231281866 (dense_cnn).

Network: conv3x3(1->6) -> Taylor-sigmoid -> conv3x3(6->7) -> flatten
         -> fc(4032->128) -> sigmoid -> fc(128->10) -> log_softmax,
batch 8192, data-parallel over 8 NeuronCores (1024 samples/core).

Mapping highlights:
  * conv2 and fc1 are adjacent linear maps -> folded on the host into one
    dense GEMM  W_comb [128, 4056] acting on the Taylor-sigmoid output.
  * conv1 is computed as a banded-weight matmul: K = input-pixel window
    (feature-major input, batch on the free dim), M = (oy, ox, oc) output
    positions.  Input is host-transposed to pixel-major [784, B] so each
    conv window is a clean strided DMA.
  * Taylor-sigmoid 1/(2 - h + h^2/2 - h^3/6 + h^4/24) is ONE custom DVE op
    (Horner quartic q = X^4+4X^3+12X^2+24X with the conv bias folded in as a
    per-partition scalar) plus ONE ScalarE Reciprocal(q/24 + 2).
  * fc2 output lands batch-major [128b, 10]; log_softmax runs on DVE+ACT.
"""

import os
import numpy as np

_B = 8192
_NCORES = 8
_PC = _B // _NCORES          # samples per core
_SLICE = 512                 # batch free-dim per matmul pass
_NSL = _PC // _SLICE         # slices per core

# conv1 output tiling: 26 = 3*8+2 rows, 26 = 7*3+5 cols
_OY_T = [(0, 3), (3, 3), (6, 3), (9, 3), (12, 3), (15, 3), (18, 3), (21, 3), (24, 2)]
_OX_T = [(0, 7), (7, 7), (14, 7), (21, 5)]

# Populated by test.py via environment; kernel() stashes profiling results here.
LAST_RESULTS = None


def _tiles():
    ts = []
    for (oy0, noy) in _OY_T:
        for (ox0, nox) in _OX_T:
            ts.append(dict(oy0=oy0, noy=noy, ox0=ox0, nox=nox,
                           ky=noy + 2, kx=nox + 2,
                           K=(noy + 2) * (nox + 2), M=noy * nox * 6,
                           cls=(noy, nox)))
    return ts


def _host_prep(x, w1, b1, w2, b2, fw1, fb1, fw2, fb2):
    """All small-weight transforms + input transpose, in numpy."""
    x = np.asarray(x, np.float32)
    w1 = np.asarray(w1, np.float32); b1 = np.asarray(b1, np.float32)
    w2 = np.asarray(w2, np.float32); b2 = np.asarray(b2, np.float32)
    fw1 = np.asarray(fw1, np.float32); fb1 = np.asarray(fb1, np.float32)
    fw2 = np.asarray(fw2, np.float32); fb2 = np.asarray(fb2, np.float32)

    tiles = _tiles()

    # --- banded conv1 weights (negated: PSUM holds -conv1(x)) per class ---
    w1b = {}
    biasv = {}
    for t in tiles:
        cls = t["cls"]
        if cls in w1b:
            continue
        noy, nox = cls
        ky, kx = noy + 2, nox + 2
        K, M = ky * kx, noy * nox * 6
        lhsT = np.zeros((K, M), np.float32)
        bv = np.zeros((M, 1), np.float32)
        for oy in range(noy):
            for ox in range(nox):
                for oc in range(6):
                    m = (oy * nox + ox) * 6 + oc
                    bv[m, 0] = -0.5 * b1[oc]
                    for dy in range(3):
                        for dx in range(3):
                            k = (oy + dy) * kx + (ox + dx)
                            lhsT[k, m] = -0.5 * w1[oc, 0, dy, dx]
        w1b[cls] = lhsT
        biasv[cls] = bv

    # --- fold conv2 + fc1 into W_comb [128, 6*26*26], b_comb [128] ---
    fw1r = fw1.reshape(128, 7, 24, 24)
    Wc = np.zeros((128, 6, 26, 26), np.float32)
    for dy in range(3):
        for dx in range(3):
            Wc[:, :, dy:dy + 24, dx:dx + 24] += np.einsum(
                "joyx,oi->jiyx", fw1r, w2[:, :, dy, dx], optimize=True)
    b_comb = fb1 + np.einsum("joyx,o->j", fw1r, b2)

    # --- W_comb columns permuted into conv1-chunk partition order ---
    Wc_flat = Wc.reshape(128, 6 * 26 * 26)
    chunks = []
    for t in tiles:
        rows = []
        for oy in range(t["noy"]):
            for ox in range(t["nox"]):
                for oc in range(6):
                    y, xx = t["oy0"] + oy, t["ox0"] + ox
                    rows.append((oc * 26 + y) * 26 + xx)
        chunks.append(np.ascontiguousarray(1.5 * Wc_flat[:, rows].T))  # [M_t, 128]
    offs = np.cumsum([0] + [c.shape[0] for c in chunks])
    wcomb = np.concatenate(chunks, axis=0)                         # [4056, 128]

    # pack per-class conv1 weights [45, 360] and bias vecs [128, 4];
    # pack W_comb chunks column-blockwise into [128, 36*128]
    cls_list = [(3, 7), (3, 5), (2, 7), (2, 5)]
    cls_off, cls_idx, o = {}, {}, 0
    for ci, cls in enumerate(cls_list):
        cls_idx[cls] = ci
        cls_off[cls] = o
        o += cls[0] * cls[1] * 6
    w1pack = np.zeros((45, o), np.float32)
    biaspack = np.zeros((128, len(cls_list)), np.float32)
    for cls in cls_list:
        K, M = w1b[cls].shape
        w1pack[:K, cls_off[cls]:cls_off[cls] + M] = w1b[cls]
        biaspack[:M, cls_idx[cls]] = biasv[cls][:, 0]
    wcpack = np.zeros((128, 128 * len(chunks)), np.float32)
    for t, ch in enumerate(chunks):
        wcpack[:ch.shape[0], 128 * t:128 * t + 128] = ch

    consts = dict(
        wcpack=wcpack, w1pack=w1pack, biaspack=biaspack,
        cls_off=cls_off, cls_idx=cls_idx,
        bcomb=b_comb.reshape(128, 1).astype(np.float32),
        fw2t=np.ascontiguousarray(fw2.T).astype(np.float32),       # [128, 10]
        fb2r=np.tile(fb2.reshape(1, 10), (128, 1)).astype(np.float32),
    )

    # --- input: pixel-major [784, B] ---
    x_pm = np.ascontiguousarray(x.reshape(_B, 784).T)
    return x_pm, consts, tiles


# ---------------------------------------------------------------------------
# custom DVE op: q(X) = X^4 + 4X^3 + 12X^2 + 24X,  X = in0 + s0(per-partition)
# ---------------------------------------------------------------------------
def _register_taylor_den16():
    import concourse.dve_ops as dve_ops
    if "TAYLOR_DEN16_ANT" in dve_ops._SUB_OPCODE_FOR_NAME:
        return next(o for o in dve_ops.OPS if o.name == "TAYLOR_DEN16_ANT")

    from concourse.dve_spec import Spec, Src0, C0, C1, C2

    # u = in0 + s0;  out = u^4 + 2u^3 + 3u^2 + 3u + 3  ==  (den(t)*24 + ...)/16
    u = Src0 + C0
    body = ((((u + C1) * u + C2) * u + C2) * u + C2)

    def _ref(in0, in1, s0, s1, imm2):
        xx = in0.astype(np.float32) + s0
        return (((xx + s1) * xx + imm2) * xx + imm2) * xx + imm2

    op = dve_ops.DveOp(
        "TAYLOR_DEN16_ANT",
        Spec(body=body, reference=_ref),
        subdim=False,
        uops_sha={"v3": "0d84493259836d20", "v4": "be052b2c26b42830"},
    )
    dve_ops.OPS.append(op)
    dve_ops.CUSTOM_DVE_SPECS[op.name] = op.spec
    row = max(dve_ops._SUB_OPCODE_FOR_NAME.values()) + 1
    assert row < 0x20
    dve_ops._SUB_OPCODE_FOR_NAME[op.name] = row
    return op


def _build_program(tiles, cls_off, cls_idx):
    import concourse.bacc as bacc
    import concourse.mybir as mybir
    from concourse.tile import TileContext
    from concourse.alu_op_type import AluOpType

    f32 = mybir.dt.float32
    AF = mybir.ActivationFunctionType
    taylor_den = _register_taylor_den16()

    nc = bacc.Bacc()
    n_tiles = len(tiles)
    xpm = nc.declare_dram_parameter("xpm", [784, _PC], f32, isOutput=False)
    wcpack_d = nc.declare_dram_parameter("wcpack", [128, 128 * n_tiles], f32,
                                         isOutput=False)
    w1pack_d = nc.declare_dram_parameter("w1pack", [45, 360], f32, isOutput=False)
    biaspack_d = nc.declare_dram_parameter("biaspack", [128, 4], f32, isOutput=False)
    bcomb_d = nc.declare_dram_parameter("bcomb", [128, 1], f32, isOutput=False)
    fw2t_d = nc.declare_dram_parameter("fw2t", [128, 10], f32, isOutput=False)
    fb2r_d = nc.declare_dram_parameter("fb2r", [128, 10], f32, isOutput=False)
    out_d = nc.declare_dram_parameter("out", [_PC, 10], f32, isOutput=True)

    with TileContext(nc) as tc:
        with (
            tc.tile_pool(name="const", bufs=1) as cpool,
            tc.tile_pool(name="xw", bufs=6) as xpool,
            tc.tile_pool(name="work", bufs=4) as wpool,
            tc.tile_pool(name="cps", bufs=3, space="PSUM") as cps,
            tc.tile_pool(name="zps", bufs=2, space="PSUM") as zps,
            tc.tile_pool(name="fps", bufs=2, space="PSUM") as fps,
        ):
            # resident constants (each one DMA; PE queue-observers below)
            w1pack_sb = cpool.tile_from(w1pack_d[:], name="w1pack_sb")
            wcpack_sb = cpool.tile_from(wcpack_d[:], name="wcpack_sb")
            biaspack_sb = cpool.tile_from(biaspack_d[:], name="biaspack_sb")
            bcomb_sb = cpool.tile_from(bcomb_d[:], name="bcomb_sb")
            fw2t_sb = cpool.tile_from(fw2t_d[:], name="fw2t_sb")
            fb2r_sb = cpool.tile_from(fb2r_d[:], name="fb2r_sb")

            # PE can carry only one sync-wait per matmul (walrus LW struct),
            # so pre-observe each PE-read const's DMA queue with a dummy
            # 1-column matmul whose lhsT and rhs come from the same tile.
            dps = fps.tile([128, 1], f32, tag="dps", name="dps", bufs=1)
            nc.tensor.matmul(dps[0:126, 0:1], w1pack_sb[0:45, 0:126],
                             w1pack_sb[0:45, 0:1], start=True, stop=True)
            nc.tensor.matmul(dps[0:128, 0:1], wcpack_sb[0:128, 0:128],
                             wcpack_sb[0:128, 0:1], start=True, stop=True)
            nc.tensor.matmul(dps[0:10, 0:1], fw2t_sb[0:128, 0:10],
                             fw2t_sb[0:128, 0:1], start=True, stop=True)
            # same single-wait rule applies to DVE custom-ISA ops and ACT:
            # pre-observe the remaining const queues on their consumer engines
            dvescr = wpool.tile([128, 14], f32, tag="dvescr", name="dvescr", bufs=1)
            nc.vector.tensor_copy(out=dvescr[:, 0:4], in_=biaspack_sb[:])
            nc.vector.tensor_copy(out=dvescr[:, 4:14], in_=fb2r_sb[:])
            actscr = wpool.tile([128, 1], f32, tag="actscr", name="actscr", bufs=1)
            nc.scalar.copy(out=actscr[:], in_=bcomb_sb[:])

            xr = xpm[:].rearrange("(h w) b -> h w b", h=28)
            zs = []
            # ---- phase A: conv1 + taylor-sigmoid + folded GEMM, both slices
            for sl in range(_NSL):
                z = zps.tile([128, _SLICE], f32, tag="z", name=f"z{sl}")
                zs.append(z)
                for i, t in enumerate(tiles):
                    xw = xpool.tile([t["K"], _SLICE], f32, tag="xw", name=f"xw{sl}_{i}")
                    nc.sync.dma_start(
                        out=xw.rearrange("(a b) n -> a b n", a=t["ky"]),
                        in_=xr[t["oy0"]:t["oy0"] + t["ky"],
                               t["ox0"]:t["ox0"] + t["kx"],
                               sl * _SLICE:(sl + 1) * _SLICE])
                    cp = cps.tile([t["M"], _SLICE], f32, tag="cp", name=f"cp{sl}_{i}")
                    co = cls_off[t["cls"]]
                    nc.tensor.matmul(cp, w1pack_sb[0:t["K"], co:co + t["M"]], xw,
                                     start=True, stop=True)
                    q = wpool.tile([t["M"], _SLICE], f32, tag="q", name=f"q{sl}_{i}")
                    ci = cls_idx[t["cls"]]
                    nc.vector._custom_dve(
                        taylor_den, out=q, in0=cp,
                        s0=biaspack_sb[0:t["M"], ci:ci + 1], s1=2.0, imm2=3.0)
                    s = wpool.tile([t["M"], _SLICE], f32, tag="s", name=f"s{sl}_{i}")
                    nc.vector.reciprocal_approx_fast(s, q)
                    nc.tensor.matmul(z, wcpack_sb[0:t["M"], 128 * i:128 * i + 128],
                                     s, start=(i == 0), stop=(i == len(tiles) - 1))
            # ---- phase B: sigmoid + fc2 + log_softmax, both slices
            for sl in range(_NSL):
                h = wpool.tile([128, _SLICE], f32, tag="h", name=f"h{sl}")
                nc.scalar.activation(h, zs[sl], AF.Sigmoid, bias=bcomb_sb[:], scale=1.0)
                ot = wpool.tile([128, (_SLICE // 128) * 10], f32, tag="ot",
                                name=f"ot{sl}")
                for g in range(_SLICE // 128):
                    fp = fps.tile([128, 10], f32, tag="fp", name=f"fp{sl}_{g}")
                    nc.tensor.matmul(fp, h[:, g * 128:(g + 1) * 128], fw2t_sb[:],
                                     start=True, stop=True)
                    lg = wpool.tile([128, 10], f32, tag="lg", name=f"lg{sl}_{g}")
                    nc.vector.tensor_tensor(out=lg, in0=fp, in1=fb2r_sb[:],
                                            op=AluOpType.add)
                    mneg = wpool.tile([128, 1], f32, tag="mn", name=f"mn{sl}_{g}")
                    nc.vector.tensor_reduce(mneg, lg, axis=mybir.AxisListType.X,
                                            op=AluOpType.max, negate=True)
                    e = wpool.tile([128, 10], f32, tag="e", name=f"e{sl}_{g}")
                    nc.scalar.activation(e, lg, AF.Exp, bias=mneg, scale=1.0)
                    ssum = wpool.tile([128, 1], f32, tag="ss", name=f"ss{sl}_{g}")
                    nc.vector.tensor_reduce(ssum, e, axis=mybir.AxisListType.X,
                                            op=AluOpType.add)
                    lns = wpool.tile([128, 1], f32, tag="ls", name=f"ls{sl}_{g}")
                    nc.scalar.activation(lns, ssum, AF.Ln)
                    nc.vector.tensor_scalar(out=ot[:, g * 10:(g + 1) * 10], in0=lg,
                                            scalar1=mneg, scalar2=lns,
                                            op0=AluOpType.add, op1=AluOpType.subtract)
                orow = sl * _SLICE
                nc.sync.dma_start(
                    out=out_d[orow:orow + _SLICE, :].rearrange(
                        "(g p) k -> p g k", p=128),
                    in_=ot.rearrange("p (g k) -> p g k", k=10))
    # Bacc.compile(): moves excess matmul waits onto ldweights, converts
    # over-capacity semaphore waits to event semaphores, inserts ACT table
    # loads, and populates .instr bytes for InstISA subclasses.
    nc.compile()
    return nc


_PROGRAM_CACHE = {}


def kernel(x, w1, b1, w2, b2, fw1, fb1, fw2, fb2):
    global LAST_RESULTS
    x_pm, consts, tiles = _host_prep(x, w1, b1, w2, b2, fw1, fb1, fw2, fb2)

    if "nc" not in _PROGRAM_CACHE:
        _PROGRAM_CACHE["nc"] = _build_program(tiles, consts["cls_off"],
                                              consts["cls_idx"])
    nc = _PROGRAM_CACHE["nc"]

    shared = {
        "wcpack": consts["wcpack"], "w1pack": consts["w1pack"],
        "biaspack": consts["biaspack"], "bcomb": consts["bcomb"],
        "fw2t": consts["fw2t"], "fb2r": consts["fb2r"],
    }

    in_maps = []
    for c in range(_NCORES):
        m = dict(shared)
        m["xpm"] = np.ascontiguousarray(x_pm[:, c * _PC:(c + 1) * _PC])
        in_maps.append(m)

    from concourse.bass_utils import run_bass_kernel_spmd
    trace = bool(int(os.environ.get("BASS_KERNEL_TRACE", "0")))
    res = run_bass_kernel_spmd(nc, in_maps, core_ids=list(range(_NCORES)),
                               trace=trace)
    LAST_RESULTS = res
    return np.concatenate([r["out"] for r in res.results], axis=0)


# revision 9
# speedup vs baseline: 2.2692x; 2.2692x over previous
"""Trainium2 Bass kernel for nn_Net_39230231281866 (dense_cnn).

Network: conv3x3(1->6) -> Taylor-sigmoid -> conv3x3(6->7) -> flatten
         -> fc(4032->128) -> sigmoid -> fc(128->10) -> log_softmax,
batch 8192, data-parallel over 8 NeuronCores (1024 samples/core).

Mapping highlights:
  * conv2 and fc1 are adjacent linear maps -> folded on the host into one
    dense GEMM  W_comb [128, 4056] acting on the Taylor-sigmoid output.
  * conv1 is computed as a banded-weight matmul: K = input-pixel window
    (feature-major input, batch on the free dim), M = (oy, ox, oc) output
    positions.  Input is host-transposed to pixel-major [784, B] so each
    conv window is a clean strided DMA.
  * Taylor-sigmoid 1/(2 - h + h^2/2 - h^3/6 + h^4/24) is ONE custom DVE op
    (Horner quartic q = X^4+4X^3+12X^2+24X with the conv bias folded in as a
    per-partition scalar) plus ONE ScalarE Reciprocal(q/24 + 2).
  * fc2 output lands batch-major [128b, 10]; log_softmax runs on DVE+ACT.
"""

import os
import numpy as np

_B = 8192
_NCORES = 8
_PC = _B // _NCORES          # samples per core
_SLICE = 512                 # batch free-dim per matmul pass
_NSL = _PC // _SLICE         # slices per core

# conv1 output tiling: 26 = 3*8+2 rows, 26 = 7*3+5 cols
_OY_T = [(0, 3), (3, 3), (6, 3), (9, 3), (12, 3), (15, 3), (18, 3), (21, 3), (24, 2)]
_OX_T = [(0, 7), (7, 7), (14, 7), (21, 5)]

# Populated by test.py via environment; kernel() stashes profiling results here.
LAST_RESULTS = None


def _tiles():
    ts = []
    for (oy0, noy) in _OY_T:
        for (ox0, nox) in _OX_T:
            ts.append(dict(oy0=oy0, noy=noy, ox0=ox0, nox=nox,
                           ky=noy + 2, kx=nox + 2,
                           K=(noy + 2) * (nox + 2), M=noy * nox * 6,
                           cls=(noy, nox)))
    return ts


def _host_prep(x, w1, b1, w2, b2, fw1, fb1, fw2, fb2):
    """All small-weight transforms + input transpose, in numpy."""
    x = np.asarray(x, np.float32)
    w1 = np.asarray(w1, np.float32); b1 = np.asarray(b1, np.float32)
    w2 = np.asarray(w2, np.float32); b2 = np.asarray(b2, np.float32)
    fw1 = np.asarray(fw1, np.float32); fb1 = np.asarray(fb1, np.float32)
    fw2 = np.asarray(fw2, np.float32); fb2 = np.asarray(fb2, np.float32)

    tiles = _tiles()

    # --- banded conv1 weights (negated: PSUM holds -conv1(x)) per class ---
    w1b = {}
    biasv = {}
    for t in tiles:
        cls = t["cls"]
        if cls in w1b:
            continue
        noy, nox = cls
        ky, kx = noy + 2, nox + 2
        K, M = ky * kx, noy * nox * 6
        lhsT = np.zeros((K, M), np.float32)
        bv = np.zeros((M, 1), np.float32)
        for oy in range(noy):
            for ox in range(nox):
                for oc in range(6):
                    m = (oy * nox + ox) * 6 + oc
                    bv[m, 0] = -0.5 * b1[oc]
                    for dy in range(3):
                        for dx in range(3):
                            k = (oy + dy) * kx + (ox + dx)
                            lhsT[k, m] = -0.5 * w1[oc, 0, dy, dx]
        w1b[cls] = lhsT
        biasv[cls] = bv

    # --- fold conv2 + fc1 into W_comb [128, 6*26*26], b_comb [128] ---
    fw1r = fw1.reshape(128, 7, 24, 24)
    Wc = np.zeros((128, 6, 26, 26), np.float32)
    for dy in range(3):
        for dx in range(3):
            Wc[:, :, dy:dy + 24, dx:dx + 24] += np.einsum(
                "joyx,oi->jiyx", fw1r, w2[:, :, dy, dx], optimize=True)
    b_comb = fb1 + np.einsum("joyx,o->j", fw1r, b2)

    # --- W_comb columns permuted into conv1-chunk partition order ---
    Wc_flat = Wc.reshape(128, 6 * 26 * 26)
    chunks = []
    for t in tiles:
        rows = []
        for oy in range(t["noy"]):
            for ox in range(t["nox"]):
                for oc in range(6):
                    y, xx = t["oy0"] + oy, t["ox0"] + ox
                    rows.append((oc * 26 + y) * 26 + xx)
        chunks.append(np.ascontiguousarray(1.5 * Wc_flat[:, rows].T))  # [M_t, 128]
    offs = np.cumsum([0] + [c.shape[0] for c in chunks])
    wcomb = np.concatenate(chunks, axis=0)                         # [4056, 128]

    # pack per-class conv1 weights [45, 360] and bias vecs [128, 4];
    # pack W_comb chunks column-blockwise into [128, 36*128]
    cls_list = [(3, 7), (3, 5), (2, 7), (2, 5)]
    cls_off, cls_idx, o = {}, {}, 0
    for ci, cls in enumerate(cls_list):
        cls_idx[cls] = ci
        cls_off[cls] = o
        o += cls[0] * cls[1] * 6
    w1pack = np.zeros((45, o), np.float32)
    biaspack = np.zeros((128, len(cls_list)), np.float32)
    for cls in cls_list:
        K, M = w1b[cls].shape
        w1pack[:K, cls_off[cls]:cls_off[cls] + M] = w1b[cls]
        biaspack[:M, cls_idx[cls]] = biasv[cls][:, 0]
    wcpack = np.zeros((128, 128 * len(chunks)), np.float32)
    for t, ch in enumerate(chunks):
        wcpack[:ch.shape[0], 128 * t:128 * t + 128] = ch

    consts = dict(
        wcpack=wcpack, w1pack=w1pack, biaspack=biaspack,
        cls_off=cls_off, cls_idx=cls_idx,
        bcomb=b_comb.reshape(128, 1).astype(np.float32),
        fw2t=np.ascontiguousarray(fw2.T).astype(np.float32),       # [128, 10]
        fb2r=np.tile(fb2.reshape(1, 10), (128, 1)).astype(np.float32),
    )

    # --- input: pixel-major [784, B] ---
    x_pm = np.ascontiguousarray(x.reshape(_B, 784).T)
    return x_pm, consts, tiles


# ---------------------------------------------------------------------------
# custom DVE op: q(X) = X^4 + 4X^3 + 12X^2 + 24X,  X = in0 + s0(per-partition)
# ---------------------------------------------------------------------------
def _register_taylor_den16():
    import concourse.dve_ops as dve_ops
    if "TAYLOR_DEN16_ANT" in dve_ops._SUB_OPCODE_FOR_NAME:
        return next(o for o in dve_ops.OPS if o.name == "TAYLOR_DEN16_ANT")

    from concourse.dve_spec import Spec, Src0, C0, C1, C2

    # u = in0 + s0;  out = u^4 + 2u^3 + 3u^2 + 3u + 3  ==  (den(t)*24 + ...)/16
    u = Src0 + C0
    body = ((((u + C1) * u + C2) * u + C2) * u + C2)

    def _ref(in0, in1, s0, s1, imm2):
        xx = in0.astype(np.float32) + s0
        return (((xx + s1) * xx + imm2) * xx + imm2) * xx + imm2

    op = dve_ops.DveOp(
        "TAYLOR_DEN16_ANT",
        Spec(body=body, reference=_ref),
        subdim=False,
        uops_sha={"v3": "0d84493259836d20", "v4": "be052b2c26b42830"},
    )
    dve_ops.OPS.append(op)
    dve_ops.CUSTOM_DVE_SPECS[op.name] = op.spec
    row = max(dve_ops._SUB_OPCODE_FOR_NAME.values()) + 1
    assert row < 0x20
    dve_ops._SUB_OPCODE_FOR_NAME[op.name] = row
    return op


def _build_program(tiles, cls_off, cls_idx):
    import concourse.bacc as bacc
    import concourse.mybir as mybir
    from concourse.tile import TileContext
    from concourse.alu_op_type import AluOpType

    f32 = mybir.dt.float32
    AF = mybir.ActivationFunctionType
    taylor_den = _register_taylor_den16()

    nc = bacc.Bacc()
    n_tiles = len(tiles)
    xpm = nc.declare_dram_parameter("xpm", [784, _PC], f32, isOutput=False)
    wcpack_d = nc.declare_dram_parameter("wcpack", [128, 128 * n_tiles], f32,
                                         isOutput=False)
    w1pack_d = nc.declare_dram_parameter("w1pack", [45, 360], f32, isOutput=False)
    biaspack_d = nc.declare_dram_parameter("biaspack", [128, 4], f32, isOutput=False)
    bcomb_d = nc.declare_dram_parameter("bcomb", [128, 1], f32, isOutput=False)
    fw2t_d = nc.declare_dram_parameter("fw2t", [128, 10], f32, isOutput=False)
    fb2r_d = nc.declare_dram_parameter("fb2r", [128, 10], f32, isOutput=False)
    out_d = nc.declare_dram_parameter("out", [_PC, 10], f32, isOutput=True)

    with TileContext(nc) as tc:
        with (
            tc.tile_pool(name="const", bufs=1) as cpool,
            tc.tile_pool(name="xw", bufs=6) as xpool,
            tc.tile_pool(name="work", bufs=4) as wpool,
            tc.tile_pool(name="cps", bufs=3, space="PSUM") as cps,
            tc.tile_pool(name="zps", bufs=2, space="PSUM") as zps,
            tc.tile_pool(name="fps", bufs=2, space="PSUM") as fps,
        ):
            # resident constants (each one DMA; PE queue-observers below)
            w1pack_sb = cpool.tile_from(w1pack_d[:], name="w1pack_sb")
            wcpack_sb = cpool.tile_from(wcpack_d[:], name="wcpack_sb")
            biaspack_sb = cpool.tile_from(biaspack_d[:], name="biaspack_sb")
            bcomb_sb = cpool.tile_from(bcomb_d[:], name="bcomb_sb")
            fw2t_sb = cpool.tile_from(fw2t_d[:], name="fw2t_sb")
            fb2r_sb = cpool.tile_from(fb2r_d[:], name="fb2r_sb")

            # PE can carry only one sync-wait per matmul (walrus LW struct),
            # so pre-observe each PE-read const's DMA queue with a dummy
            # 1-column matmul whose lhsT and rhs come from the same tile.
            dps = fps.tile([128, 1], f32, tag="dps", name="dps", bufs=1)
            nc.tensor.matmul(dps[0:126, 0:1], w1pack_sb[0:45, 0:126],
                             w1pack_sb[0:45, 0:1], start=True, stop=True)
            nc.tensor.matmul(dps[0:128, 0:1], wcpack_sb[0:128, 0:128],
                             wcpack_sb[0:128, 0:1], start=True, stop=True)
            nc.tensor.matmul(dps[0:10, 0:1], fw2t_sb[0:128, 0:10],
                             fw2t_sb[0:128, 0:1], start=True, stop=True)
            # same single-wait rule applies to DVE custom-ISA ops and ACT:
            # pre-observe the remaining const queues on their consumer engines
            dvescr = wpool.tile([128, 14], f32, tag="dvescr", name="dvescr", bufs=1)
            nc.vector.tensor_copy(out=dvescr[:, 0:4], in_=biaspack_sb[:])
            nc.vector.tensor_copy(out=dvescr[:, 4:14], in_=fb2r_sb[:])
            actscr = wpool.tile([128, 1], f32, tag="actscr", name="actscr", bufs=1)
            nc.scalar.copy(out=actscr[:], in_=bcomb_sb[:])

            xr = xpm[:].rearrange("(h w) b -> h w b", h=28)
            zs = []
            # ---- phase A: conv1 + taylor-sigmoid + folded GEMM, both slices
            for sl in range(_NSL):
                z = zps.tile([128, _SLICE], f32, tag="z", name=f"z{sl}")
                zs.append(z)
                for i, t in enumerate(tiles):
                    xw = xpool.tile([t["K"], _SLICE], f32, tag="xw", name=f"xw{sl}_{i}")
                    nc.sync.dma_start(
                        out=xw,
                        in_=xr[t["oy0"]:t["oy0"] + t["ky"],
                               t["ox0"]:t["ox0"] + t["kx"],
                               sl * _SLICE:(sl + 1) * _SLICE])
                    cp = cps.tile([t["M"], _SLICE], f32, tag="cp", name=f"cp{sl}_{i}")
                    co = cls_off[t["cls"]]
                    nc.tensor.matmul(cp, w1pack_sb[0:t["K"], co:co + t["M"]], xw,
                                     start=True, stop=True)
                    q = wpool.tile([t["M"], _SLICE], f32, tag="q", name=f"q{sl}_{i}")
                    ci = cls_idx[t["cls"]]
                    nc.vector._custom_dve(
                        taylor_den, out=q, in0=cp,
                        s0=biaspack_sb[0:t["M"], ci:ci + 1], s1=2.0, imm2=3.0)
                    s = wpool.tile([t["M"], _SLICE], f32, tag="s", name=f"s{sl}_{i}")
                    nc.vector.reciprocal_approx_fast(s, q)
                    nc.tensor.matmul(z, wcpack_sb[0:t["M"], 128 * i:128 * i + 128],
                                     s, start=(i == 0), stop=(i == len(tiles) - 1))
            # ---- phase B: sigmoid + fc2 + log_softmax, both slices
            for sl in range(_NSL):
                h = wpool.tile([128, _SLICE], f32, tag="h", name=f"h{sl}")
                nc.scalar.activation(h, zs[sl], AF.Sigmoid, bias=bcomb_sb[:], scale=1.0)
                ot = wpool.tile([128, (_SLICE // 128) * 10], f32, tag="ot",
                                name=f"ot{sl}")
                for g in range(_SLICE // 128):
                    fp = fps.tile([128, 10], f32, tag="fp", name=f"fp{sl}_{g}")
                    nc.tensor.matmul(fp, h[:, g * 128:(g + 1) * 128], fw2t_sb[:],
                                     start=True, stop=True)
                    lg = wpool.tile([128, 10], f32, tag="lg", name=f"lg{sl}_{g}")
                    nc.vector.tensor_tensor(out=lg, in0=fp, in1=fb2r_sb[:],
                                            op=AluOpType.add)
                    mneg = wpool.tile([128, 1], f32, tag="mn", name=f"mn{sl}_{g}")
                    nc.vector.tensor_reduce(mneg, lg, axis=mybir.AxisListType.X,
                                            op=AluOpType.max, negate=True)
                    e = wpool.tile([128, 10], f32, tag="e", name=f"e{sl}_{g}")
                    nc.scalar.activation(e, lg, AF.Exp, bias=mneg, scale=1.0)
                    ssum = wpool.tile([128, 1], f32, tag="ss", name=f"ss{sl}_{g}")
                    nc.vector.tensor_reduce(ssum, e, axis=mybir.AxisListType.X,
                                            op=AluOpType.add)
                    lns = wpool.tile([128, 1], f32, tag="ls", name=f"ls{sl}_{g}")
                    nc.scalar.activation(lns, ssum, AF.Ln)
                    nc.vector.tensor_scalar(out=ot[:, g * 10:(g + 1) * 10], in0=lg,
                                            scalar1=mneg, scalar2=lns,
                                            op0=AluOpType.add, op1=AluOpType.subtract)
                orow = sl * _SLICE
                nc.sync.dma_start(
                    out=out_d[orow:orow + _SLICE, :].rearrange(
                        "(g p) k -> p g k", p=128),
                    in_=ot.rearrange("p (g k) -> p g k", k=10))
    # Bacc.compile(): moves excess matmul waits onto ldweights, converts
    # over-capacity semaphore waits to event semaphores, inserts ACT table
    # loads, and populates .instr bytes for InstISA subclasses.
    nc.compile()
    return nc


_PROGRAM_CACHE = {}


def kernel(x, w1, b1, w2, b2, fw1, fb1, fw2, fb2):
    global LAST_RESULTS
    x_pm, consts, tiles = _host_prep(x, w1, b1, w2, b2, fw1, fb1, fw2, fb2)

    if "nc" not in _PROGRAM_CACHE:
        _PROGRAM_CACHE["nc"] = _build_program(tiles, consts["cls_off"],
                                              consts["cls_idx"])
    nc = _PROGRAM_CACHE["nc"]

    shared = {
        "wcpack": consts["wcpack"], "w1pack": consts["w1pack"],
        "biaspack": consts["biaspack"], "bcomb": consts["bcomb"],
        "fw2t": consts["fw2t"], "fb2r": consts["fb2r"],
    }

    in_maps = []
    for c in range(_NCORES):
        m = dict(shared)
        m["xpm"] = np.ascontiguousarray(x_pm[:, c * _PC:(c + 1) * _PC])
        in_maps.append(m)

    from concourse.bass_utils import run_bass_kernel_spmd
    trace = bool(int(os.environ.get("BASS_KERNEL_TRACE", "0")))
    res = run_bass_kernel_spmd(nc, in_maps, core_ids=list(range(_NCORES)),
                               trace=trace)
    LAST_RESULTS = res
    return np.concatenate([r["out"] for r in res.results], axis=0)


# revision 10
# speedup vs baseline: 4.7108x; 2.0760x over previous
"""Trainium2 Bass kernel for nn_Net_39230231281866 (dense_cnn).

Network: conv3x3(1->6) -> Taylor-sigmoid -> conv3x3(6->7) -> flatten
         -> fc(4032->128) -> sigmoid -> fc(128->10) -> log_softmax,
batch 8192, data-parallel over 8 NeuronCores (1024 samples/core).

Mapping:
  * conv2+fc1 folded on the host into one dense GEMM W_comb [128, 4056].
  * conv1 = banded-weight matmul (K = input-pixel window, M = 128 padded
    output positions x channels, batch on the moving free dim). Input is
    host-transposed to pixel-major [784, B] bf16 so windows are clean DMAs.
  * Matmuls run in bf16 (fp32 is 4 cycles/row on the PE; bf16 is 1).
    PSUM accumulation stays fp32.
  * Taylor-sigmoid: custom DVE op computes den16(u) = u^4+2u^3+3u^2+3u+3
    where u = (-conv1(x)-b1)/2 (scale folded into the conv weights), then
    a Reciprocal (mostly on ScalarE via the PWP table, a few on VectorE
    via RECIPROCAL_APPROX_FAST to balance engines). s = (24/16)/den16 with
    the 1.5 folded into W_comb.
  * conv1 outputs are written in PSUM-bank pairs so elementwise ops run
    at free-dim 1024, amortizing instruction overheads.
"""

import os
import numpy as np
import ml_dtypes

_B = 8192
_NCORES = 8
_PC = _B // _NCORES
_SLICE = 512
_NSL = _PC // _SLICE

# conv1 output tiling: 26 = 3*8+2 rows, 26 = 7*3+5 cols
_OY_T = [(0, 3), (3, 3), (6, 3), (9, 3), (12, 3), (15, 3), (18, 3), (21, 3), (24, 2)]
_OX_T = [(0, 7), (7, 7), (14, 7), (21, 5)]

# how many of the 36 per-core reciprocal super-ops run on the DVE
# (the rest run on ScalarE) — engine balance knob
_N_DVE_RECIP = 4

LAST_RESULTS = None


def _tiles():
    ts = []
    for (oy0, noy) in _OY_T:
        for (ox0, nox) in _OX_T:
            ts.append(dict(oy0=oy0, noy=noy, ox0=ox0, nox=nox,
                           ky=noy + 2, kx=nox + 2,
                           K=(noy + 2) * (nox + 2), M=noy * nox * 6,
                           cls=(noy, nox)))
    return ts


def _host_prep(x, w1, b1, w2, b2, fw1, fb1, fw2, fb2):
    x = np.asarray(x, np.float32)
    w1 = np.asarray(w1, np.float32); b1 = np.asarray(b1, np.float32)
    w2 = np.asarray(w2, np.float32); b2 = np.asarray(b2, np.float32)
    fw1 = np.asarray(fw1, np.float32); fb1 = np.asarray(fb1, np.float32)
    fw2 = np.asarray(fw2, np.float32); fb2 = np.asarray(fb2, np.float32)

    tiles = _tiles()

    # banded conv1 weights, scaled by -1/2 (u = (-conv-b1)/2), M padded to 128
    cls_list = [(3, 7), (3, 5), (2, 7), (2, 5)]
    cls_idx = {c: i for i, c in enumerate(cls_list)}
    w1pack = np.zeros((45, 128 * 4), np.float32)
    biaspack = np.zeros((128, 4), np.float32)
    for cls in cls_list:
        noy, nox = cls
        kx = nox + 2
        ci = cls_idx[cls]
        for oy in range(noy):
            for ox in range(nox):
                for oc in range(6):
                    m = (oy * nox + ox) * 6 + oc
                    biaspack[m, ci] = -0.5 * b1[oc]
                    for dy in range(3):
                        for dx in range(3):
                            k = (oy + dy) * kx + (ox + dx)
                            w1pack[k, 128 * ci + m] = -0.5 * w1[oc, 0, dy, dx]

    # fold conv2 + fc1 -> W_comb [128, 6*26*26] (x1.5: s = 1.5/den16), b_comb
    fw1r = fw1.reshape(128, 7, 24, 24)
    Wc = np.zeros((128, 6, 26, 26), np.float32)
    for dy in range(3):
        for dx in range(3):
            Wc[:, :, dy:dy + 24, dx:dx + 24] += np.einsum(
                "joyx,oi->jiyx", fw1r, w2[:, :, dy, dx], optimize=True)
    b_comb = fb1 + np.einsum("joyx,o->j", fw1r, b2)
    Wc_flat = (1.5 * Wc.reshape(128, 6 * 26 * 26)).astype(np.float32)

    # W_comb columns in conv1-chunk partition order, packed [128, 36*128]
    wcpack = np.zeros((128, 128 * len(tiles)), np.float32)
    for t_i, t in enumerate(tiles):
        rows = []
        for oy in range(t["noy"]):
            for ox in range(t["nox"]):
                for oc in range(6):
                    rows.append((oc * 26 + t["oy0"] + oy) * 26 + t["ox0"] + ox)
        wcpack[:t["M"], 128 * t_i:128 * t_i + 128] = Wc_flat[:, rows].T

    bf = ml_dtypes.bfloat16
    consts = dict(
        wcpack=wcpack.astype(bf), w1pack=w1pack.astype(bf),
        biaspack=biaspack, cls_idx=cls_idx,
        bcomb=b_comb.reshape(128, 1).astype(np.float32),
        fw2t=np.ascontiguousarray(fw2.T).astype(bf),                    # [128, 10]
        fb2r=np.tile(fb2.reshape(1, 10), (128, 4)).astype(np.float32),  # [128, 40]
    )
    x_pm = np.ascontiguousarray(x.reshape(_B, 784).T.astype(bf))        # [784, B]
    return x_pm, consts, tiles


def _register_taylor_den16():
    import concourse.dve_ops as dve_ops
    if "TAYLOR_DEN16_ANT" in dve_ops._SUB_OPCODE_FOR_NAME:
        return next(o for o in dve_ops.OPS if o.name == "TAYLOR_DEN16_ANT")
    from concourse.dve_spec import Spec, Src0, C0, C1, C2

    # u = in0 + s0;  out = u^4 + 2u^3 + 3u^2 + 3u + 3  ==  (q(t)+48)/16
    u = Src0 + C0
    body = ((((u + C1) * u + C2) * u + C2) * u + C2)

    def _ref(in0, in1, s0, s1, imm2):
        xx = in0.astype(np.float32) + s0
        return (((xx + s1) * xx + imm2) * xx + imm2) * xx + imm2

    op = dve_ops.DveOp(
        "TAYLOR_DEN16_ANT",
        Spec(body=body, reference=_ref),
        subdim=False,
        uops_sha={"v3": "0d84493259836d20", "v4": "be052b2c26b42830"},
    )
    dve_ops.OPS.append(op)
    dve_ops.CUSTOM_DVE_SPECS[op.name] = op.spec
    row = max(dve_ops._SUB_OPCODE_FOR_NAME.values()) + 1
    assert row < 0x20
    dve_ops._SUB_OPCODE_FOR_NAME[op.name] = row
    return op


def _pin_exp_ln_table():
    """Make Exp and Ln resolve only to natural_log_exp_and_others so the
    log_softmax tail costs one table load instead of alternating sets."""
    import concourse.bacc as bacc
    import concourse.mybir as mybir
    if getattr(bacc, "_ant_expln_pinned", False):
        return
    orig = bacc.get_activation_tables
    AF = mybir.ActivationFunctionType

    def patched(arch):
        tabs = {k: set(v) for k, v in orig(arch).items()}
        for name, fns in tabs.items():
            if name != "natural_log_exp_and_others":
                fns.discard(AF.Exp)
                fns.discard(AF.Ln)
        return tabs

    bacc.get_activation_tables = patched
    bacc._ant_expln_pinned = True


def _act_raw(nc, out, in_, func, bias=0.0, scale=1.0):
    """Emit InstActivation directly (used for Reciprocal, which the
    nc.scalar.activation wrapper refuses; measured ~1.2e-5 rel err)."""
    import concourse.mybir as mybir
    eng = nc.scalar
    inputs = [eng.lower_ap(in_)]
    for arg in (bias, scale, 0.0):
        inputs.append(mybir.ImmediateValue(dtype=mybir.dt.float32,
                                           value=float(arg)))
    return eng.add_instruction(mybir.InstActivation(
        name=nc.get_next_instruction_name(), func=func, ins=inputs,
        outs=[eng.lower_ap(out)]))


def _build_program(tiles, cls_idx):
    import concourse.bacc as bacc
    import concourse.mybir as mybir
    from concourse.tile import TileContext
    from concourse.alu_op_type import AluOpType
    from concourse.dve_ops import RECIP_APPROX_FAST_CONSTS as RC
    import concourse.dve_ops as dve_ops

    f32 = mybir.dt.float32
    bf16 = mybir.dt.bfloat16
    AF = mybir.ActivationFunctionType
    taylor_den = _register_taylor_den16()
    recip_fast = next(o for o in dve_ops.OPS if o.name == "RECIPROCAL_APPROX_FAST")
    _pin_exp_ln_table()

    nc = bacc.Bacc()
    n_tiles = len(tiles)
    xpm = nc.declare_dram_parameter("xpm", [784, _PC], bf16, isOutput=False)
    wcpack_d = nc.declare_dram_parameter("wcpack", [128, 128 * n_tiles], bf16,
                                         isOutput=False)
    w1pack_d = nc.declare_dram_parameter("w1pack", [45, 512], bf16, isOutput=False)
    biaspack_d = nc.declare_dram_parameter("biaspack", [128, 4], f32, isOutput=False)
    bcomb_d = nc.declare_dram_parameter("bcomb", [128, 1], f32, isOutput=False)
    fw2t_d = nc.declare_dram_parameter("fw2t", [128, 10], bf16, isOutput=False)
    fb2r_d = nc.declare_dram_parameter("fb2r", [128, 40], f32, isOutput=False)
    out_d = nc.declare_dram_parameter("out", [_PC, 10], f32, isOutput=True)

    with TileContext(nc) as tc:
        with (
            tc.tile_pool(name="const", bufs=1) as cpool,
            tc.tile_pool(name="xw", bufs=8) as xpool,
            tc.tile_pool(name="work", bufs=3) as wpool,
            tc.tile_pool(name="cps", bufs=2, space="PSUM") as cps,
            tc.tile_pool(name="zps", bufs=2, space="PSUM") as zps,
            tc.tile_pool(name="fps", bufs=1, space="PSUM") as fps,
        ):
            w1pack_sb = cpool.tile_from(w1pack_d[:], name="w1pack_sb")
            wcpack_sb = cpool.tile_from(wcpack_d[:], name="wcpack_sb")
            biaspack_sb = cpool.tile_from(biaspack_d[:], name="biaspack_sb")
            bcomb_sb = cpool.tile_from(bcomb_d[:], name="bcomb_sb")
            fw2t_sb = cpool.tile_from(fw2t_d[:], name="fw2t_sb")
            fb2r_sb = cpool.tile_from(fb2r_d[:], name="fb2r_sb")

            # single-sync-wait rule: pre-observe PE-read const queues with
            # dummy 1-col matmuls; DVE/ACT-read consts with dummy touches.
            dps = fps.tile([128, 1], f32, tag="dps", name="dps", bufs=1)
            nc.tensor.matmul(dps[0:128, 0:1], w1pack_sb[0:45, 0:128],
                             w1pack_sb[0:45, 0:1], start=True, stop=True)
            nc.tensor.matmul(dps[0:128, 0:1], wcpack_sb[0:128, 0:128],
                             wcpack_sb[0:128, 0:1], start=True, stop=True)
            nc.tensor.matmul(dps[0:10, 0:1], fw2t_sb[0:128, 0:10],
                             fw2t_sb[0:128, 0:1], start=True, stop=True)
            dvescr = wpool.tile([128, 44], f32, tag="dvescr", name="dvescr", bufs=1)
            nc.vector.tensor_copy(out=dvescr[:, 0:4], in_=biaspack_sb[:])
            nc.vector.tensor_copy(out=dvescr[:, 4:44], in_=fb2r_sb[:])
            actscr = wpool.tile([128, 1], f32, tag="actscr", name="actscr", bufs=1)
            nc.scalar.copy(out=actscr[:], in_=bcomb_sb[:])

            xr = xpm[:].rearrange("(h w) b -> h w b", h=28)
            zs = []
            n_pairs = n_tiles // 2
            total_pairs = n_pairs * _NSL
            recip_on_dve = set()
            if _N_DVE_RECIP > 0:
                step = max(1, total_pairs // _N_DVE_RECIP)
                recip_on_dve = set(list(range(0, total_pairs, step))[:_N_DVE_RECIP])

            pair_seq = 0
            for sl in range(_NSL):
                z = zps.tile([128, _SLICE], f32, tag="z", name=f"z{sl}")
                zs.append(z)
                for pi in range(n_pairs):
                    ta, tb = tiles[2 * pi], tiles[2 * pi + 1]
                    cp = cps.tile([128, 2 * _SLICE], f32, tag="cp",
                                  name=f"cp{sl}_{pi}")
                    q = wpool.tile([128, 2 * _SLICE], f32, tag="q",
                                   name=f"q{sl}_{pi}")
                    s = wpool.tile([128, 2 * _SLICE], bf16, tag="s",
                                   name=f"s{sl}_{pi}")
                    for half, t in ((0, ta), (1, tb)):
                        i = 2 * pi + half
                        xw = xpool.tile([t["K"], _SLICE], bf16, tag="xw",
                                        name=f"xw{sl}_{i}")
                        nc.sync.dma_start(
                            out=xw,
                            in_=xr[t["oy0"]:t["oy0"] + t["ky"],
                                   t["ox0"]:t["ox0"] + t["kx"],
                                   sl * _SLICE:(sl + 1) * _SLICE])
                        ci = cls_idx[t["cls"]]
                        nc.tensor.matmul(
                            cp[:, half * _SLICE:(half + 1) * _SLICE],
                            w1pack_sb[0:t["K"], 128 * ci:128 * ci + 128], xw,
                            start=True, stop=True)
                    ca, cb = cls_idx[ta["cls"]], cls_idx[tb["cls"]]
                    if ca == cb:
                        nc.vector._custom_dve(
                            taylor_den, out=q, in0=cp,
                            s0=biaspack_sb[0:128, ca:ca + 1], s1=2.0, imm2=3.0)
                    else:
                        nc.vector._custom_dve(
                            taylor_den, out=q[:, 0:_SLICE], in0=cp[:, 0:_SLICE],
                            s0=biaspack_sb[0:128, ca:ca + 1], s1=2.0, imm2=3.0)
                        nc.vector._custom_dve(
                            taylor_den, out=q[:, _SLICE:2 * _SLICE],
                            in0=cp[:, _SLICE:2 * _SLICE],
                            s0=biaspack_sb[0:128, cb:cb + 1], s1=2.0, imm2=3.0)
                    if pair_seq in recip_on_dve:
                        nc.vector._custom_dve(
                            recip_fast, out=s, in0=q,
                            s0=RC["s0"], s1=RC["s1"], imm2=RC["imm2"])
                    else:
                        _act_raw(nc, s, q, AF.Reciprocal)
                    pair_seq += 1
                    for half, t in ((0, ta), (1, tb)):
                        i = 2 * pi + half
                        nc.tensor.matmul(
                            z, wcpack_sb[0:128, 128 * i:128 * i + 128],
                            s[:, half * _SLICE:(half + 1) * _SLICE],
                            start=(i == 0), stop=(i == n_tiles - 1))
            # ---- tail: sigmoid, fc2, log_softmax (no max-sub: |logits| < 12,
            # exp cannot overflow fp32) ----
            hs = []
            for sl in range(_NSL):
                h = wpool.tile([128, _SLICE], bf16, tag="h", name=f"h{sl}")
                nc.scalar.activation(h, zs[sl], AF.Sigmoid, bias=bcomb_sb[:],
                                     scale=1.0)
                hs.append(h)
            for sl in range(_NSL):
                ng = _SLICE // 128
                fp = fps.tile([128, 10 * ng], f32, tag="fp", name=f"fp{sl}", bufs=1)
                for g in range(ng):
                    nc.tensor.matmul(fp[:, g * 10:(g + 1) * 10],
                                     hs[sl][:, g * 128:(g + 1) * 128], fw2t_sb[:],
                                     start=True, stop=True)
                lg = wpool.tile([128, 10 * ng], f32, tag="lg", name=f"lg{sl}")
                nc.vector.tensor_tensor(out=lg, in0=fp, in1=fb2r_sb[:, 0:10 * ng],
                                        op=AluOpType.add)
                e = wpool.tile([128, 10 * ng], f32, tag="e", name=f"e{sl}")
                nc.scalar.activation(e, lg, AF.Exp)
                ssum = wpool.tile([128, ng], f32, tag="ss", name=f"ss{sl}")
                nc.vector.tensor_reduce(
                    ssum, e.rearrange("p (g k) -> p g k", k=10),
                    axis=mybir.AxisListType.X, op=AluOpType.add)
                lns = wpool.tile([128, ng], f32, tag="ls", name=f"ls{sl}")
                nc.scalar.activation(lns, ssum, AF.Ln)
                ot = wpool.tile([128, 10 * ng], f32, tag="ot", name=f"ot{sl}")
                for g in range(ng):
                    nc.vector.tensor_scalar(
                        out=ot[:, g * 10:(g + 1) * 10],
                        in0=lg[:, g * 10:(g + 1) * 10],
                        scalar1=lns[:, g:g + 1], scalar2=None,
                        op0=AluOpType.subtract)
                orow = sl * _SLICE
                nc.sync.dma_start(
                    out=out_d[orow:orow + _SLICE, :].rearrange(
                        "(g p) k -> p g k", p=128),
                    in_=ot.rearrange("p (g k) -> p g k", k=10))
    nc.compile()
    return nc


_PROGRAM_CACHE = {}


def kernel(x, w1, b1, w2, b2, fw1, fb1, fw2, fb2):
    global LAST_RESULTS
    x_pm, consts, tiles = _host_prep(x, w1, b1, w2, b2, fw1, fb1, fw2, fb2)

    if "nc" not in _PROGRAM_CACHE:
        _PROGRAM_CACHE["nc"] = _build_program(tiles, consts["cls_idx"])
    nc = _PROGRAM_CACHE["nc"]

    shared = {k: consts[k] for k in
              ("wcpack", "w1pack", "biaspack", "bcomb", "fw2t", "fb2r")}
    in_maps = []
    for c in range(_NCORES):
        m = dict(shared)
        m["xpm"] = np.ascontiguousarray(x_pm[:, c * _PC:(c + 1) * _PC])
        in_maps.append(m)

    from concourse.bass_utils import run_bass_kernel_spmd
    trace = bool(int(os.environ.get("BASS_KERNEL_TRACE", "0")))
    res = run_bass_kernel_spmd(nc, in_maps, core_ids=list(range(_NCORES)),
                               trace=trace)
    LAST_RESULTS = res
    return np.concatenate([r["out"] for r in res.results], axis=0)
